# revision 1
# baseline (speedup 1.0000x reference)
"""CrossFeatureFusion TRN2 kernel.

out[i] = x[i] + sum_{j != i} (x[j] @ W[i,j]^T + b[i,j])
x: [4, 65536, 256] f32, W: [4, 4, 256, 256] f32, b: [4, 4, 256] f32.

Strategy (data-parallel over N, 8 NeuronCores, no collectives):
  - Host: transpose x to feature-major shards xt[core][j, fc, k, n] so the
    contraction dim (f = fc*128 + k) lies on SBUF partitions with no
    on-chip transpose.
  - Host: build block weights M[i][j] = (I if j == i else W[i,j]^T), packed
    per output pair (0,1) / (2,3) as the moving operand.  The identity
    diagonal folds the residual "+ x[i]" into the PSUM accumulation.
  - Device: per 128-row block, 16 fp32r matmuls of [K=128] x [N=512]
    accumulate the full fused output for all 4 modalities in 2 PSUM banks;
    DVE adds the precomputed bias sums while draining PSUM -> SBUF;
    HWDGE DMAs move x-shards in and outputs back.
  - fp32r (TF32-like PE mode) runs at ~1 row/cycle for moving dim >= 256;
    measured rel err vs fp32 reference ~1.5e-4.
"""

import sys

if "/opt/trn_rl_repo" not in sys.path:
    sys.path.insert(0, "/opt/trn_rl_repo")

import numpy as np

M, N, D = 4, 65536, 256
N_CORES = 8
NSH = N // N_CORES  # rows per core
NBLK = NSH // 128  # 128-row blocks per core
PAIRS = ((0, 1), (2, 3))

_CACHE = {}


def _build_nc(nsh=NSH, repeat=1, xbufs=4, obufs=4, pbufs=4):
    from concourse import bacc
    import concourse.mybir as mybir
    import concourse.tile as tile

    f32 = mybir.dt.float32
    f32r = mybir.dt.float32r
    nblk = nsh // 128

    nc = bacc.Bacc(debug=False)
    xt_d = nc.dram_tensor("xt", [M, 2, 128, nsh], f32r, kind="ExternalInput")
    wp_d = nc.dram_tensor("wp", [2, 8, 128, 512], f32r, kind="ExternalInput")
    bb_d = nc.dram_tensor("bb", [1, 2, 512], f32, kind="ExternalInput")
    out_d = nc.dram_tensor("out", [M, nsh, D], f32, kind="ExternalOutput")

    with tile.TileContext(nc) as tc:
        with (
            tc.tile_pool(name="wsb", bufs=1) as wpool,
            tc.tile_pool(name="xt", bufs=xbufs) as xpool,
            tc.tile_pool(name="osb", bufs=obufs) as opool,
            tc.tile_pool(name="psum", bufs=pbufs, space="PSUM") as ppool,
        ):
            w_sb = wpool.tile([128, 2, 8, 512], f32r)
            nc.sync.dma_start(out=w_sb[:], in_=wp_d.rearrange("p c k e -> k p c e"))
            bias_sb = wpool.tile([128, 2, 512], f32)
            nc.sync.dma_start(
                out=bias_sb[:], in_=bb_d[:].to_broadcast([128, 2, 512])
            )

            def body():
                for nb in range(nblk):
                    n0 = nb * 128
                    xt_sb = xpool.tile([128, M, 2, 128], f32r, name="xt_sb", tag="xt")
                    nc.sync.dma_start(
                        out=xt_sb[:],
                        in_=xt_d[:, :, :, n0 : n0 + 128].rearrange(
                            "j f k n -> k j f n"
                        ),
                    )
                    pss = [
                        ppool.tile([128, 512], f32, tag=f"ps{p}", name=f"ps{p}_{nb}")
                        for p in range(2)
                    ]
                    for c in range(8):
                        j, fc = c >> 1, c & 1
                        for p in range(2):
                            nc.tensor.matmul(
                                pss[p][:],
                                lhsT=xt_sb[:, j, fc, :],
                                rhs=w_sb[:, p, c, :],
                                start=(c == 0),
                                stop=(c == 7),
                            )
                    for p in range(2):
                        o_sb = opool.tile(
                            [128, 2, 256], f32, name=f"osb{p}_{nb}", tag="osb"
                        )
                        nc.vector.tensor_add(
                            out=o_sb[:].rearrange("n i e -> n (i e)"),
                            in0=pss[p][:],
                            in1=bias_sb[:, p, :],
                        )
                        nc.sync.dma_start(
                            out=out_d[2 * p : 2 * p + 2, n0 : n0 + 128, :].rearrange(
                                "i n e -> n i e"
                            ),
                            in_=o_sb[:],
                        )

            if repeat > 1:
                with tc.For_i(0, repeat, 1):
                    body()
            else:
                body()
    nc.finalize()
    return nc


def _build_nc_v2(nsh=NSH, repeat=1, xbufs=3, obufs=6, pbufs=2):
    """out^T formulation: W stationary, xt moving -> PSUM holds out^T[i]
    chunks [128 e, 512 n].  No identity matmuls: the residual "+x[i]" is a
    direct DVE add from the (already transposed) xt tile, fused with the
    bias add in one scalar_tensor_tensor while draining PSUM.  Host
    un-transposes the [4, 2, 128, nsh] output during gather."""
    from concourse import bacc
    import concourse.mybir as mybir
    import concourse.tile as tile

    f32 = mybir.dt.float32
    f32r = mybir.dt.float32r
    NB = 512  # rows per block
    nblk = nsh // NB
    add = mybir.AluOpType.add

    nc = bacc.Bacc(debug=False)
    xt_d = nc.dram_tensor("xt", [M, 2, 128, nsh], f32r, kind="ExternalInput")
    wst_d = nc.dram_tensor("wst", [8, 6, 128, 128], f32r, kind="ExternalInput")
    bbt_d = nc.dram_tensor("bbt", [8, 128], f32, kind="ExternalInput")
    out_d = nc.dram_tensor("out", [M, 2, 128, nsh], f32, kind="ExternalOutput")

    jl = [[j for j in range(M) if j != i] for i in range(M)]

    with tile.TileContext(nc) as tc:
        with (
            tc.tile_pool(name="wsb", bufs=1) as wpool,
            tc.tile_pool(name="xt", bufs=xbufs) as xpool,
            tc.tile_pool(name="osb", bufs=obufs) as opool,
            tc.tile_pool(name="psum", bufs=pbufs, space="PSUM") as ppool,
        ):
            w_sb = wpool.tile([128, 8, 6, 128], f32r)
            nc.sync.dma_start(out=w_sb[:], in_=wst_d.rearrange("t c k m -> k t c m"))
            bias_sb = wpool.tile([128, 8], f32)
            nc.sync.dma_start(out=bias_sb[:], in_=bbt_d.rearrange("t k -> k t"))

            def body():
                for nb in range(nblk):
                    n0 = nb * NB
                    xt_sb = xpool.tile([128, M, 2, NB], f32r, name="xt_sb", tag="xt")
                    nc.sync.dma_start(
                        out=xt_sb[:],
                        in_=xt_d[:, :, :, n0 : n0 + NB].rearrange(
                            "j f k n -> k j f n"
                        ),
                    )
                    for half in range(2):
                        pss = [
                            ppool.tile(
                                [128, NB], f32, tag=f"ps{t}", name=f"ps{t}_{nb}"
                            )
                            for t in range(4)
                        ]
                        for tt in range(4):
                            tg = half * 4 + tt
                            i = tg >> 1
                            for cc in range(6):
                                fc = cc & 1
                                j = jl[i][cc >> 1]
                                nc.tensor.matmul(
                                    pss[tt][:],
                                    lhsT=w_sb[:, tg, cc, :],
                                    rhs=xt_sb[:, j, fc, :],
                                    start=(cc == 0),
                                    stop=(cc == 5),
                                )
                        for tt in range(4):
                            tg = half * 4 + tt
                            i, ec = tg >> 1, tg & 1
                            o_sb = opool.tile(
                                [128, NB], f32, name=f"osb{tg}_{nb}", tag="osb"
                            )
                            nc.vector.scalar_tensor_tensor(
                                out=o_sb[:],
                                in0=pss[tt][:],
                                scalar=bias_sb[:, tg : tg + 1],
                                in1=xt_sb[:, i, ec, :].bitcast(f32),
                                op0=add,
                                op1=add,
                            )
                            nc.sync.dma_start(
                                out=out_d[i, ec, :, n0 : n0 + NB], in_=o_sb[:]
                            )

            if repeat > 1:
                with tc.For_i(0, repeat, 1):
                    body()
            else:
                body()
    nc.finalize()
    return nc


def _get_exec(**build_kwargs):
    """Build (once per config) the jitted 8-core executor. Returns a callable
    run(xt_g, wst_g, bbt_g, n_iters) -> out_g with global concat arrays."""
    key = tuple(sorted(build_kwargs.items()))
    if key in _CACHE:
        return _CACHE[key]

    import jax
    import jax.numpy as jnp
    from jax.sharding import Mesh, PartitionSpec
    from jax.experimental.shard_map import shard_map
    from concourse import bass2jax

    nc = _build_nc_v2(**build_kwargs)
    bass2jax.install_neuronx_cc_hook()

    in_names = ["xt", "wst", "bbt", "out"]
    if nc.partition_id_tensor is not None:
        in_names.append(nc.partition_id_tensor.name)
    out_names = ["out"]
    out_aval = jax.core.ShapedArray((M, 2, 128, NSH), np.float32)

    def _body(xt, wst, bbt, out_zero):
        operands = [xt, wst, bbt, out_zero]
        if nc.partition_id_tensor is not None:
            operands.append(bass2jax.partition_id_tensor())
        outs = bass2jax._bass_exec_p.bind(
            *operands,
            out_avals=(out_aval,),
            in_names=tuple(in_names),
            out_names=tuple(out_names),
            lowering_input_output_aliases=(),
            sim_require_finite=True,
            sim_require_nnan=True,
            nc=nc,
        )
        return tuple(outs)

    devices = jax.devices()[:N_CORES]
    mesh = Mesh(np.asarray(devices), ("core",))
    sharded = jax.jit(
        shard_map(
            _body,
            mesh=mesh,
            in_specs=(PartitionSpec("core"),) * 4,
            out_specs=(PartitionSpec("core"),),
            check_rep=False,
        ),
        donate_argnums=(3,),
        keep_unused=True,
    )

    sharding = jax.sharding.NamedSharding(mesh, PartitionSpec("core"))
    zeros_fn = jax.jit(
        lambda: jnp.zeros((N_CORES * M, 2, 128, NSH), np.float32),
        out_shardings=sharding,
    )

    def run(xt_g, wst_g, bbt_g, n_iters=1):
        xt_j = jax.device_put(xt_g, sharding)
        wst_j = jax.device_put(wst_g, sharding)
        bbt_j = jax.device_put(bbt_g, sharding)
        outs = None
        for _ in range(n_iters):
            outs = sharded(xt_j, wst_j, bbt_j, zeros_fn())
        jax.block_until_ready(outs)
        return outs[0]

    _CACHE[key] = run
    return run


def _prep_inputs(x, W, b):
    """Host-side shard + layout prep. Returns global concatenated arrays."""
    x = np.asarray(x, dtype=np.float32)
    W = np.asarray(W, dtype=np.float32)
    b = np.asarray(b, dtype=np.float32)
    n = x.shape[1]
    nsh = n // N_CORES

    # xt_g[(c*M + j), fc, k, n] = x[j, c*nsh + n, fc*128 + k]
    x4 = x.reshape(M, N_CORES, nsh, D)
    xt_g = np.ascontiguousarray(x4.transpose(1, 0, 3, 2)).reshape(
        N_CORES * M, 2, 128, nsh
    )

    # Stationary W chunks: wst[(i*2+ec), cc, k, m] = W[i, jl[cc>>1]].T block
    wst = np.empty((8, 6, 128, 128), dtype=np.float32)
    for i in range(M):
        jli = [j for j in range(M) if j != i]
        for ec in range(2):
            t = i * 2 + ec
            for cc in range(6):
                j = jli[cc >> 1]
                fc = cc & 1
                wst[t, cc] = W[i, j][
                    ec * 128 : (ec + 1) * 128, fc * 128 : (fc + 1) * 128
                ].T
    wst_g = np.ascontiguousarray(
        np.broadcast_to(wst[None], (N_CORES, 8, 6, 128, 128))
    ).reshape(N_CORES * 8, 6, 128, 128)

    # bias sums: BS[i] = sum_{j != i} b[i, j];  bbt[(i*2+ec), k]
    bs = b.sum(axis=1) - b[np.arange(M), np.arange(M)]  # [4, 256]
    bbt = bs.reshape(8, 128)
    bbt_g = np.ascontiguousarray(
        np.broadcast_to(bbt[None], (N_CORES, 8, 128))
    ).reshape(N_CORES * 8, 128)

    return xt_g, wst_g, bbt_g


def kernel(x, W, b):
    xt_g, wst_g, bbt_g = _prep_inputs(x, W, b)
    run = _get_exec()
    out_g = run(xt_g, wst_g, bbt_g)
    # out_g: [NC*M, 2, 128, NSH]; out[j, c*NSH+n, ec*128+m] = out_g[c*4+j, ec, m, n]
    out = np.asarray(out_g).reshape(N_CORES, M, 2, 128, NSH)
    out = np.ascontiguousarray(out.transpose(1, 0, 4, 2, 3)).reshape(M, N, D)
    return out



# revision 2
# speedup vs baseline: 30.4905x; 30.4905x over previous
"""CrossFeatureFusion TRN2 kernel.

out[i] = x[i] + sum_{j != i} (x[j] @ W[i,j]^T + b[i,j])
x: [4, 65536, 256] f32, W: [4, 4, 256, 256] f32, b: [4, 4, 256] f32.

Strategy (data-parallel over N, 8 NeuronCores, no collectives):
  - Host: cast x/W to bf16 (measured end-to-end rel err ~3e-3, gate 2e-2)
    and lay out per-core inputs block-major so every DMA is a single
    contiguous 8KB-per-partition transfer:
      xt[nb*128+k, j, fc, n] = x[j, n0+n, fc*128+k]   (out^T formulation)
  - Device: per 512-row block, 48 bf16 matmuls [K=128]x[N=512] accumulate
    out^T[i] chunks for all 8 (i, e-half) targets in 8 PSUM banks; DVE
    fuses (+bias, +residual x[i]) while draining PSUM -> SBUF bf16; one
    1MB DMA writes the block's 8 targets back.
  - PE is the roofline: 768 MM x 512 rows @ 1 row/cycle ~ 164us/pass.
    bf16 halves HBM traffic vs f32 (~35MB/core/pass) so DMA hides under
    the matmuls.
"""

import sys

if "/opt/trn_rl_repo" not in sys.path:
    sys.path.insert(0, "/opt/trn_rl_repo")

import numpy as np
import ml_dtypes

BF16 = ml_dtypes.bfloat16

M, N, D = 4, 65536, 256
N_CORES = 8
NSH = N // N_CORES  # rows per core
NB = 512  # rows per block (one PSUM bank of f32)
NBLK = NSH // NB  # blocks per core

_CACHE = {}


def _build_nc(nsh=NSH, repeat=1, xbufs=4, obufs=3, pbufs=2):
    """out^T formulation: W stationary (bf16), xt moving -> PSUM f32 holds
    out^T[i] chunks [128 e, 512 n].  DVE drains PSUM with fused
    (+bias, +residual) into a per-block [128, 8, 512] bf16 tile; a single
    DMA stores it.  All DRAM layouts are per-partition contiguous."""
    from concourse import bacc
    import concourse.mybir as mybir
    import concourse.tile as tile

    f32 = mybir.dt.float32
    bf16 = mybir.dt.bfloat16
    nblk = nsh // NB
    add = mybir.AluOpType.add

    nc = bacc.Bacc(debug=False)
    xt_d = nc.dram_tensor("xt", [nblk * 128, M, 2, NB], bf16, kind="ExternalInput")
    wst_d = nc.dram_tensor("wst", [128, 8, 6, 128], bf16, kind="ExternalInput")
    bbt_d = nc.dram_tensor("bbt", [128, 8], f32, kind="ExternalInput")
    out_d = nc.dram_tensor("out", [nblk * 128, 8, NB], bf16, kind="ExternalOutput")

    jl = [[j for j in range(M) if j != i] for i in range(M)]

    with tile.TileContext(nc) as tc:
        with (
            tc.tile_pool(name="wsb", bufs=1) as wpool,
            tc.tile_pool(name="xt", bufs=xbufs) as xpool,
            tc.tile_pool(name="osb", bufs=obufs) as opool,
            tc.tile_pool(name="psum", bufs=pbufs, space="PSUM") as ppool,
        ):
            w_sb = wpool.tile([128, 8, 6, 128], bf16)
            nc.sync.dma_start(out=w_sb[:], in_=wst_d[:])
            bias_sb = wpool.tile([128, 8], f32)
            nc.sync.dma_start(out=bias_sb[:], in_=bbt_d[:])

            def body():
                for nb in range(nblk):
                    r0 = nb * 128
                    xt_sb = xpool.tile([128, M, 2, NB], bf16, name="xt_sb", tag="xt")
                    nc.sync.dma_start(out=xt_sb[:], in_=xt_d[r0 : r0 + 128])
                    o_sb = opool.tile([128, 8, NB], bf16, name=f"osb_{nb}", tag="osb")
                    for half in range(2):
                        pss = [
                            ppool.tile(
                                [128, NB], f32, tag=f"ps{t}", name=f"ps{t}_{nb}"
                            )
                            for t in range(4)
                        ]
                        for tt in range(4):
                            tg = half * 4 + tt
                            i = tg >> 1
                            for cc in range(6):
                                fc = cc & 1
                                j = jl[i][cc >> 1]
                                nc.tensor.matmul(
                                    pss[tt][:],
                                    lhsT=w_sb[:, tg, cc, :],
                                    rhs=xt_sb[:, j, fc, :],
                                    start=(cc == 0),
                                    stop=(cc == 5),
                                )
                        for tt in range(4):
                            tg = half * 4 + tt
                            i, ec = tg >> 1, tg & 1
                            nc.vector.scalar_tensor_tensor(
                                out=o_sb[:, tg, :],
                                in0=pss[tt][:],
                                scalar=bias_sb[:, tg : tg + 1],
                                in1=xt_sb[:, i, ec, :],
                                op0=add,
                                op1=add,
                            )
                    nc.sync.dma_start(out=out_d[r0 : r0 + 128], in_=o_sb[:])

            if repeat > 1:
                with tc.For_i(0, repeat, 1):
                    body()
            else:
                body()
    nc.finalize()
    return nc


def _get_exec(**build_kwargs):
    """Build (once per config) the jitted 8-core executor.  Returns a callable
    run(xt_g, wst_g, bbt_g, n_iters) -> out_g; also exposes run.put() /
    run.exec_() so callers can keep transfers out of timed regions."""
    key = tuple(sorted(build_kwargs.items()))
    if key in _CACHE:
        return _CACHE[key]

    import jax
    import jax.numpy as jnp
    from jax.sharding import Mesh, PartitionSpec
    from jax.experimental.shard_map import shard_map
    from concourse import bass2jax

    nc = _build_nc(**build_kwargs)
    bass2jax.install_neuronx_cc_hook()

    in_names = ["xt", "wst", "bbt", "out"]
    if nc.partition_id_tensor is not None:
        in_names.append(nc.partition_id_tensor.name)
    out_aval = jax.core.ShapedArray((NBLK * 128, 8, NB), jnp.bfloat16)

    def _body(xt, wst, bbt, out_zero):
        operands = [xt, wst, bbt, out_zero]
        if nc.partition_id_tensor is not None:
            operands.append(bass2jax.partition_id_tensor())
        outs = bass2jax._bass_exec_p.bind(
            *operands,
            out_avals=(out_aval,),
            in_names=tuple(in_names),
            out_names=("out",),
            lowering_input_output_aliases=(),
            sim_require_finite=True,
            sim_require_nnan=True,
            nc=nc,
        )
        return tuple(outs)

    devices = jax.devices()[:N_CORES]
    mesh = Mesh(np.asarray(devices), ("core",))
    sharded = jax.jit(
        shard_map(
            _body,
            mesh=mesh,
            in_specs=(PartitionSpec("core"),) * 4,
            out_specs=(PartitionSpec("core"),),
            check_rep=False,
        ),
        keep_unused=True,
    )
    sharding = jax.sharding.NamedSharding(mesh, PartitionSpec("core"))

    def put(xt_g, wst_g, bbt_g):
        z = jax.device_put(
            np.zeros((N_CORES * NBLK * 128, 8, NB), BF16), sharding
        )
        handles = (
            jax.device_put(xt_g, sharding),
            jax.device_put(wst_g, sharding),
            jax.device_put(bbt_g, sharding),
            z,
        )
        jax.block_until_ready(handles)
        return handles

    def exec_(handles):
        outs = sharded(*handles)
        jax.block_until_ready(outs)
        return outs[0]

    def run(xt_g, wst_g, bbt_g, n_iters=1):
        handles = put(xt_g, wst_g, bbt_g)
        outs = None
        for _ in range(n_iters):
            outs = exec_(handles)
        return outs

    run.put = put
    run.exec_ = exec_
    _CACHE[key] = run
    return run


def _prep_inputs(x, W, b):
    """Host-side shard + layout prep.  Returns global concatenated arrays."""
    x = np.asarray(x, dtype=np.float32)
    W = np.asarray(W, dtype=np.float32)
    b = np.asarray(b, dtype=np.float32)
    n = x.shape[1]
    nsh = n // N_CORES
    nblk = nsh // NB

    # xt_g[(c*nblk + nb)*128 + k, j, fc, nn] = x[j, c*nsh + nb*NB + nn, fc*128 + k]
    xb = x.astype(BF16)
    x6 = xb.reshape(M, N_CORES, nblk, NB, 2, 128)  # j, c, nb, nn, fc, k
    xt_g = np.ascontiguousarray(x6.transpose(1, 2, 5, 0, 4, 3)).reshape(
        N_CORES * nblk * 128, M, 2, NB
    )

    # Stationary W chunks, k-major: wkm[k, (i*2+ec), cc, m] = W[i,j].T block
    wkm = np.empty((128, 8, 6, 128), dtype=np.float32)
    for i in range(M):
        jli = [j for j in range(M) if j != i]
        for ec in range(2):
            t = i * 2 + ec
            for cc in range(6):
                j = jli[cc >> 1]
                fc = cc & 1
                wkm[:, t, cc, :] = W[i, j][
                    ec * 128 : (ec + 1) * 128, fc * 128 : (fc + 1) * 128
                ].T
    wkm = wkm.astype(BF16)
    wst_g = np.ascontiguousarray(
        np.broadcast_to(wkm[None], (N_CORES, 128, 8, 6, 128))
    ).reshape(N_CORES * 128, 8, 6, 128)

    # bias sums (f32): BS[i] = sum_{j != i} b[i, j]; bbt[k, i*2+ec] = BS[i, ec*128+k]
    bs = b.sum(axis=1) - b[np.arange(M), np.arange(M)]  # [4, 256]
    bbt = np.ascontiguousarray(
        bs.reshape(M * 2, 128).T  # [128, 8]
    ).astype(np.float32)
    bbt_g = np.ascontiguousarray(
        np.broadcast_to(bbt[None], (N_CORES, 128, 8))
    ).reshape(N_CORES * 128, 8)

    return xt_g, wst_g, bbt_g


def _gather_out(out_g):
    """out_g: [NC*NBLK*128, 8, NB] bf16 ->
    out[i, c*NSH + nb*NB + nn, ec*128 + k] = out_g[(c*NBLK+nb)*128+k, i*2+ec, nn]"""
    og = np.asarray(out_g).reshape(N_CORES, NBLK, 128, M, 2, NB)
    out = og.transpose(3, 0, 1, 5, 4, 2)  # i, c, nb, nn, ec, k
    return np.ascontiguousarray(out, dtype=np.float32).reshape(M, N, D)


def kernel(x, W, b):
    xt_g, wst_g, bbt_g = _prep_inputs(x, W, b)
    run = _get_exec()
    out_g = run(xt_g, wst_g, bbt_g)
    return _gather_out(out_g)


# revision 3
# speedup vs baseline: 47.4643x; 1.5567x over previous
"""CrossFeatureFusion TRN2 kernel.

out[i] = x[i] + sum_{j != i} (x[j] @ W[i,j]^T + b[i,j])
x: [4, 65536, 256] f32, W: [4, 4, 256, 256] f32, b: [4, 4, 256] f32.

Strategy (data-parallel over N, 8 NeuronCores, no collectives):
  - Host: cast x/W to bf16 (measured end-to-end rel err ~3e-3, gate 2e-2)
    and lay out per-core inputs block-major so every DMA is a single
    contiguous 8KB-per-partition transfer:
      xt[nb*128+k, j, fc, n] = x[j, n0+n, fc*128+k]   (out^T formulation)
  - Device: per 512-row block, 48 bf16 matmuls [K=128]x[N=512] accumulate
    out^T[i] chunks for all 8 (i, e-half) targets in 8 PSUM banks; DVE
    fuses (+bias, +residual x[i]) while draining PSUM -> SBUF bf16; one
    1MB DMA writes the block's 8 targets back.
  - PE is the roofline: 768 MM x 512 rows @ 1 row/cycle ~ 164us/pass.
    bf16 halves HBM traffic vs f32 (~35MB/core/pass) so DMA hides under
    the matmuls.
"""

import sys

if "/opt/trn_rl_repo" not in sys.path:
    sys.path.insert(0, "/opt/trn_rl_repo")

import numpy as np
import ml_dtypes

BF16 = ml_dtypes.bfloat16

M, N, D = 4, 65536, 256
N_CORES = 8
NSH = N // N_CORES  # rows per core
NB = 512  # rows per block (one PSUM bank of f32)
NBLK = NSH // NB  # blocks per core

_CACHE = {}


def _build_nc(nsh=NSH, repeat=1, xbufs=6, obufs=4, pbufs=2):
    """out^T formulation: W stationary (bf16), xt moving -> PSUM f32 holds
    out^T[i] chunks [128 e, 512 n].  DVE drains PSUM with fused
    (+bias, +residual) into a per-block [128, 8, 512] bf16 tile; a single
    DMA stores it.  All DRAM layouts are per-partition contiguous."""
    from concourse import bacc
    import concourse.mybir as mybir
    import concourse.tile as tile

    f32 = mybir.dt.float32
    bf16 = mybir.dt.bfloat16
    nblk = nsh // NB
    add = mybir.AluOpType.add

    nc = bacc.Bacc(debug=False)
    xt_d = nc.dram_tensor("xt", [nblk * 128, M, 2, NB], bf16, kind="ExternalInput")
    wst_d = nc.dram_tensor("wst", [128, 8, 6, 128], bf16, kind="ExternalInput")
    bbt_d = nc.dram_tensor("bbt", [128, 8], f32, kind="ExternalInput")
    out_d = nc.dram_tensor("out", [nblk * 128, 8, NB], bf16, kind="ExternalOutput")

    jl = [[j for j in range(M) if j != i] for i in range(M)]

    with tile.TileContext(nc) as tc:
        with (
            tc.tile_pool(name="wsb", bufs=1) as wpool,
            tc.tile_pool(name="xt", bufs=xbufs) as xpool,
            tc.tile_pool(name="osb", bufs=obufs) as opool,
            tc.tile_pool(name="psum", bufs=pbufs, space="PSUM") as ppool,
        ):
            w_sb = wpool.tile([128, 8, 6, 128], bf16)
            nc.sync.dma_start(out=w_sb[:], in_=wst_d[:])
            bias_sb = wpool.tile([128, 8], f32)
            nc.sync.dma_start(out=bias_sb[:], in_=bbt_d[:])

            def body():
                for nb in range(nblk):
                    r0 = nb * 128
                    xt_sb = xpool.tile([128, M, 2, NB], bf16, name="xt_sb", tag="xt")
                    nc.sync.dma_start(out=xt_sb[:], in_=xt_d[r0 : r0 + 128])
                    o_sb = opool.tile([128, 8, NB], bf16, name=f"osb_{nb}", tag="osb")
                    for half in range(2):
                        pss = [
                            ppool.tile(
                                [128, NB], f32, tag=f"ps{t}", name=f"ps{t}_{nb}"
                            )
                            for t in range(4)
                        ]
                        for tt in range(4):
                            tg = half * 4 + tt
                            i = tg >> 1
                            for cc in range(6):
                                fc = cc & 1
                                j = jl[i][cc >> 1]
                                nc.tensor.matmul(
                                    pss[tt][:],
                                    lhsT=w_sb[:, tg, cc, :],
                                    rhs=xt_sb[:, j, fc, :],
                                    start=(cc == 0),
                                    stop=(cc == 5),
                                )
                        for tt in range(4):
                            tg = half * 4 + tt
                            i, ec = tg >> 1, tg & 1
                            nc.vector.scalar_tensor_tensor(
                                out=o_sb[:, tg, :],
                                in0=pss[tt][:],
                                scalar=bias_sb[:, tg : tg + 1],
                                in1=xt_sb[:, i, ec, :],
                                op0=add,
                                op1=add,
                            )
                    nc.sync.dma_start(out=out_d[r0 : r0 + 128], in_=o_sb[:])

            if repeat > 1:
                with tc.For_i(0, repeat, 1):
                    body()
            else:
                body()
    nc.finalize()
    return nc


def _get_exec(**build_kwargs):
    """Build (once per config) the jitted 8-core executor.  Returns a callable
    run(xt_g, wst_g, bbt_g, n_iters) -> out_g; also exposes run.put() /
    run.exec_() so callers can keep transfers out of timed regions."""
    key = tuple(sorted(build_kwargs.items()))
    if key in _CACHE:
        return _CACHE[key]

    import jax
    import jax.numpy as jnp
    from jax.sharding import Mesh, PartitionSpec
    from jax.experimental.shard_map import shard_map
    from concourse import bass2jax

    nc = _build_nc(**build_kwargs)
    bass2jax.install_neuronx_cc_hook()

    in_names = ["xt", "wst", "bbt", "out"]
    if nc.partition_id_tensor is not None:
        in_names.append(nc.partition_id_tensor.name)
    out_aval = jax.core.ShapedArray((NBLK * 128, 8, NB), jnp.bfloat16)

    def _body(xt, wst, bbt, out_zero):
        operands = [xt, wst, bbt, out_zero]
        if nc.partition_id_tensor is not None:
            operands.append(bass2jax.partition_id_tensor())
        outs = bass2jax._bass_exec_p.bind(
            *operands,
            out_avals=(out_aval,),
            in_names=tuple(in_names),
            out_names=("out",),
            lowering_input_output_aliases=(),
            sim_require_finite=True,
            sim_require_nnan=True,
            nc=nc,
        )
        return tuple(outs)

    devices = jax.devices()[:N_CORES]
    mesh = Mesh(np.asarray(devices), ("core",))
    sharded = jax.jit(
        shard_map(
            _body,
            mesh=mesh,
            in_specs=(PartitionSpec("core"),) * 4,
            out_specs=(PartitionSpec("core"),),
            check_rep=False,
        ),
        keep_unused=True,
    )
    sharding = jax.sharding.NamedSharding(mesh, PartitionSpec("core"))

    def put(xt_g, wst_g, bbt_g):
        z = jax.device_put(
            np.zeros((N_CORES * NBLK * 128, 8, NB), BF16), sharding
        )
        handles = (
            jax.device_put(xt_g, sharding),
            jax.device_put(wst_g, sharding),
            jax.device_put(bbt_g, sharding),
            z,
        )
        jax.block_until_ready(handles)
        return handles

    def exec_(handles):
        outs = sharded(*handles)
        jax.block_until_ready(outs)
        return outs[0]

    def run(xt_g, wst_g, bbt_g, n_iters=1):
        handles = put(xt_g, wst_g, bbt_g)
        outs = None
        for _ in range(n_iters):
            outs = exec_(handles)
        return outs

    run.put = put
    run.exec_ = exec_
    _CACHE[key] = run
    return run


def _prep_inputs(x, W, b):
    """Host-side shard + layout prep.  Returns global concatenated arrays."""
    x = np.asarray(x, dtype=np.float32)
    W = np.asarray(W, dtype=np.float32)
    b = np.asarray(b, dtype=np.float32)
    n = x.shape[1]
    nsh = n // N_CORES
    nblk = nsh // NB

    # xt_g[(c*nblk + nb)*128 + k, j, fc, nn] = x[j, c*nsh + nb*NB + nn, fc*128 + k]
    xb = x.astype(BF16)
    x6 = xb.reshape(M, N_CORES, nblk, NB, 2, 128)  # j, c, nb, nn, fc, k
    xt_g = np.ascontiguousarray(x6.transpose(1, 2, 5, 0, 4, 3)).reshape(
        N_CORES * nblk * 128, M, 2, NB
    )

    # Stationary W chunks, k-major: wkm[k, (i*2+ec), cc, m] = W[i,j].T block
    wkm = np.empty((128, 8, 6, 128), dtype=np.float32)
    for i in range(M):
        jli = [j for j in range(M) if j != i]
        for ec in range(2):
            t = i * 2 + ec
            for cc in range(6):
                j = jli[cc >> 1]
                fc = cc & 1
                wkm[:, t, cc, :] = W[i, j][
                    ec * 128 : (ec + 1) * 128, fc * 128 : (fc + 1) * 128
                ].T
    wkm = wkm.astype(BF16)
    wst_g = np.ascontiguousarray(
        np.broadcast_to(wkm[None], (N_CORES, 128, 8, 6, 128))
    ).reshape(N_CORES * 128, 8, 6, 128)

    # bias sums (f32): BS[i] = sum_{j != i} b[i, j]; bbt[k, i*2+ec] = BS[i, ec*128+k]
    bs = b.sum(axis=1) - b[np.arange(M), np.arange(M)]  # [4, 256]
    bbt = np.ascontiguousarray(
        bs.reshape(M * 2, 128).T  # [128, 8]
    ).astype(np.float32)
    bbt_g = np.ascontiguousarray(
        np.broadcast_to(bbt[None], (N_CORES, 128, 8))
    ).reshape(N_CORES * 128, 8)

    return xt_g, wst_g, bbt_g


def _gather_out(out_g):
    """out_g: [NC*NBLK*128, 8, NB] bf16 ->
    out[i, c*NSH + nb*NB + nn, ec*128 + k] = out_g[(c*NBLK+nb)*128+k, i*2+ec, nn]"""
    og = np.asarray(out_g).reshape(N_CORES, NBLK, 128, M, 2, NB)
    out = og.transpose(3, 0, 1, 5, 4, 2)  # i, c, nb, nn, ec, k
    return np.ascontiguousarray(out, dtype=np.float32).reshape(M, N, D)


def kernel(x, W, b):
    xt_g, wst_g, bbt_g = _prep_inputs(x, W, b)
    run = _get_exec()
    out_g = run(xt_g, wst_g, bbt_g)
    return _gather_out(out_g)
